# revision 10
# baseline (speedup 1.0000x reference)
"""Deformable 3D conv net on 8 Trainium2 NeuronCores (Bass/Tile).

Sharding: core (b, q) = batch b in {0,1} x D-quarter q in {0..3};
each core computes out[b, :, 12q:12q+12, :, :] from a padded x slab.

Per-core algorithm (exact trilinear, 5-wide window, exact for |off|<=2;
offsets clamped to [-2,2] on device; actual max |off| ~ 1.83):
  1. PE off-conv: off[81, 48,48] per d-slice, contraction K=96
     (3 w-shift replicas x 32 channels) accumulated over 9 (kd,kh) taps.
  2. Per tap k: zeta[(dd,dh,dw), h, w] = hat(od-dd)hat(oh-dh)hat(ow-dw)
     built with doubling copies + tensor_scalar chains (bf16, 125 rows).
  3. Per channel c: xr = 125 delta-shifted replicas of padded x channel c
     (one replicating DMA, double-buffered so c+1's load hides under c's
     compute); per tap: P = zeta * xr_window (DVE bf16); PE matmul K=125
     with stationary w_dc[o,c,k] broadcast over rows accumulates
     out[32, h, w] in PSUM across all (k, c).

Perf notes: offsets round-trip DRAM in bf16 (halves the 125-row zeta
broadcast-read traffic); hat tensors are computed in-place in the three
per-axis bc tiles (saves SBUF for xr double-buffering); output is
evicted bf16 and widened to f32 on host.
"""

import numpy as np
import ml_dtypes

import concourse.bass as bass
import concourse.bacc as bacc
import concourse.mybir as mybir
from concourse.tile import TileContext
from concourse.bass_utils import run_bass_kernel_spmd

B, C, O, S = 2, 32, 32, 48
KS, KV = 3, 27
PAD = 4
DP = 12                 # output D per core
DPP = DP + 2 * PAD      # 20
HP = WP = S + 2 * PAD   # 56
HWP = HP * WP           # 3136
NPAD = DPP * HWP        # 62720
NDELTA = 125

F32 = mybir.dt.float32
BF16 = mybir.dt.bfloat16
ALU = mybir.AluOpType
ACTF = mybir.ActivationFunctionType

TAP_GROUPS = [list(range(0, 14)), list(range(14, 27))]
HCHUNKS = [(0, 10), (10, 10), (20, 10), (30, 10), (40, 8)]  # h-row chunks
NS_LOOP = DP  # number of d-slices traced (reduce for simulation tests)
LAST_RESULTS = None


# ---------------------------------------------------------------- host prep
def _build_core_inputs(x, w_off, b_off, w_dc, b_dc, b, q):
    xp = np.zeros((C, DPP, HP, WP), np.float32)
    d0 = DP * q - PAD
    lo, hi = max(0, -d0), min(DPP, S - d0)
    xp[:, lo:hi, PAD:PAD + S, PAD:PAD + S] = x[b, :, d0 + lo:d0 + hi]

    # x3[32g+c, d, h, w] = xp[c, d, h, w + (g-1)]  (wrap lands in zero pad)
    x3 = np.zeros((96, DPP, HP, WP), np.float32)
    for g in range(3):
        x3[32 * g:32 * g + 32] = np.roll(xp, -(g - 1), axis=3)
    x3 = x3.reshape(96, NPAD).astype(ml_dtypes.bfloat16)

    x_bf = xp.reshape(C, NPAD).astype(ml_dtypes.bfloat16)

    # w_off9: [9*96, 81]: chunk (kd,kh), rows (kw, c), cols m = 3k + axis
    woff = w_off.reshape(KV, 3, C, KS, KS, KS)
    w_off9 = np.zeros((9, 96, 81), np.float32)
    for kd in range(3):
        for kh in range(3):
            ch = kd * 3 + kh
            for kw in range(3):
                blk = woff[:, :, :, kd, kh, kw]          # (k, ax, c)
                w_off9[ch, 32 * kw:32 * kw + 32, :] = \
                    blk.transpose(2, 0, 1).reshape(C, KV * 3)
    w_off9 = w_off9.astype(ml_dtypes.bfloat16)

    # wdc_rep: [128, KV*C*O]: rows = delta (125 used), free (k, c, o)
    wdcf = w_dc.reshape(O, C, KV)
    wdc = np.zeros((128, KV * C * O), np.float32)
    wdc[:NDELTA, :] = wdcf.transpose(2, 1, 0).reshape(KV * C * O)[None, :]
    wdc = wdc.astype(ml_dtypes.bfloat16)

    dd = np.repeat(np.arange(-2, 3), 25).astype(np.float32)[:, None]
    dh = np.tile(np.repeat(np.arange(-2, 3), 5), 5).astype(np.float32)[:, None]
    dw = np.tile(np.arange(-2, 3), 25).astype(np.float32)[:, None]

    return {
        "x3": np.ascontiguousarray(x3),
        "x_bf": np.ascontiguousarray(x_bf),
        "w_off9": np.ascontiguousarray(w_off9.transpose(1, 0, 2).reshape(96, 9 * 81)),
        "wdc_rep": np.ascontiguousarray(wdc),
        "b_off": np.ascontiguousarray(b_off.astype(np.float32).reshape(81, 1)),
        "b_dc": np.ascontiguousarray(b_dc.astype(np.float32).reshape(32, 1)),
        "dvec_d": dd, "dvec_h": dh, "dvec_w": dw,
    }


# ---------------------------------------------------------------- device IR
def _win_ap(dram_row_ap, offset, ap_dims):
    a = dram_row_ap.copy()
    a.ap = mybir.VecI64Pair(ap_dims)
    a.offset = offset
    return a


def _build_zeta(nc, pool, ds, k, off_dram, dvecs, zeta, hats, bcs):
    """hat(ax) = max(1 - |off_row - dvec|, 0) on 125 rows; zeta = prod."""
    for ax in range(3):
        bc = bcs[ax]
        src = _win_ap(off_dram[0:1, :],
                      (ds * 81 + 3 * k + ax) * S * S,
                      [(0, NDELTA), (1, S * S)])
        nc.sync.dma_start(bc.rearrange("p h w -> p (h w)"), src)
        # u = |dvec - bc| ; h = relu(1 - u)   (both on the scalar engine)
        nc.scalar.activation(bc[:], bc[:], ACTF.Abs,
                             bias=dvecs[ax][:, :], scale=-1.0)
        nc.scalar.activation(bc[:], bc[:], ACTF.Relu,
                             bias=1.0, scale=-1.0)
    nc.vector.tensor_tensor(zeta[:], bcs[0][:], bcs[1][:], ALU.mult)
    nc.vector.tensor_tensor(zeta[:], zeta[:], bcs[2][:], ALU.mult)


OFF_DT = BF16  # offsets round-trip DRAM in bf16: halves the zeta DMA bytes


def build_kernel(nc: bass.Bass):
    x3_d = nc.dram_tensor("x3", [96, NPAD], BF16, kind="ExternalInput")
    xbf_d = nc.dram_tensor("x_bf", [C, NPAD], BF16, kind="ExternalInput")
    woff_d = nc.dram_tensor("w_off9", [96, 9 * 81], BF16, kind="ExternalInput")
    wdc_d = nc.dram_tensor("wdc_rep", [128, KV * C * O], BF16,
                           kind="ExternalInput")
    boff_d = nc.dram_tensor("b_off", [81, 1], F32, kind="ExternalInput")
    bdc_d = nc.dram_tensor("b_dc", [32, 1], F32, kind="ExternalInput")
    dv_d = [nc.dram_tensor(n, [NDELTA, 1], F32, kind="ExternalInput")
            for n in ("dvec_d", "dvec_h", "dvec_w")]
    off_dram = nc.dram_tensor("off_scratch", [1, NS_LOOP * 81 * S * S], OFF_DT,
                              kind="Internal")
    out_d = nc.dram_tensor("out", [O, NS_LOOP * S * S], BF16,
                           kind="ExternalOutput")

    with TileContext(nc) as tc:
        with tc.tile_pool(name="fixed", bufs=1) as fixed:
            woff_s = fixed.tile([96, 9 * 81], BF16)
            nc.sync.dma_start(woff_s[:, :], woff_d[:, :])
            wdc_s = fixed.tile([128, KV * C * O], BF16)
            nc.sync.dma_start(wdc_s[:, :], wdc_d[:, :])
            boff_s = fixed.tile([81, 1], F32)
            nc.sync.dma_start(boff_s[:, :], boff_d[:, :])
            bdc_s = fixed.tile([32, 1], F32)
            nc.sync.dma_start(bdc_s[:, :], bdc_d[:, :])
            dvecs = []
            for i, t in enumerate(dv_d):
                dv = fixed.tile([NDELTA, 1], F32, name=f"dv{i}")
                nc.sync.dma_start(dv[:, :], t[:, :])
                dvecs.append(dv)

            # warm fixed tiles on DVE once so later DVE instructions don't
            # each carry a DMA-sem wait (HW wait-slot limit)
            warm = fixed.tile([1, 8], F32)
            for wsrc in [boff_s, bdc_s] + dvecs:
                nc.vector.tensor_copy(warm[0:1, 0:1], wsrc[0:1, 0:1])

            for ds in range(NS_LOOP):
                _do_offconv(nc, tc, ds, x3_d, off_dram, woff_s, boff_s)
            for ds in range(NS_LOOP):
                _do_slice(nc, tc, ds, x3_d, xbf_d, out_d, off_dram,
                          woff_s, wdc_s, boff_s, bdc_s, dvecs)
    return nc


def _do_offconv(nc, tc, ds, x3_d, off_dram, woff_s, boff_s):
    dpad = ds + PAD
    with tc.tile_pool(name=f"oc{ds}", bufs=1) as pool, \
         tc.tile_pool(name=f"ocps{ds}", bufs=1, space="PSUM") as psp:
        x3s = pool.tile([96, 3, HP, WP], BF16, name=f"x3s{ds}", tag="x3s",
                        bufs=2)
        nc.sync.dma_start(
            x3s.rearrange("p a h w -> p (a h w)"),
            x3_d[:, (dpad - 1) * HWP:(dpad + 2) * HWP])
        off = pool.tile([81, S, S], OFF_DT, name=f"off{ds}", tag="off",
                        bufs=2)
        for hc, (hb, hn) in enumerate(HCHUNKS):
            ps = psp.tile([81, hn, S], F32, name=f"offps{ds}_{hc}",
                          tag="offps", bufs=2)
            for i in range(9):
                kd, kh = i // 3, i % 3
                rhs = x3s[:, kd, 3 + kh + hb:3 + kh + hb + hn, 4:52]
                nc.tensor.matmul(ps[:], woff_s[:, i * 81:(i + 1) * 81],
                                 rhs, start=(i == 0), stop=(i == 8))
            # evict + bias + clamp to [-2, 2]
            nc.vector.tensor_scalar(off[:, hb:hb + hn, :], ps[:],
                                    boff_s[:, :], 2.0, ALU.add, ALU.min)
        nc.vector.tensor_scalar(off[:], off[:], -2.0, None, ALU.max)
        nc.sync.dma_start(
            _win_ap(off_dram[0:1, :], ds * 81 * S * S,
                    [(S * S, 81), (1, S * S)]),
            off.rearrange("p h w -> p (h w)"))


def _do_slice(nc, tc, ds, x3_d, xbf_d, out_d, off_dram,
              woff_s, wdc_s, boff_s, bdc_s, dvecs):
    dpad = ds + PAD
    with tc.tile_pool(name=f"sl{ds}", bufs=1) as pool, \
         tc.tile_pool(name=f"psum{ds}", bufs=1, space="PSUM") as psp:
        # ---------------- accumulators ----------------
        accs = [psp.tile([O, hn, S], F32, name=f"acc{ds}_{ci}", tag=f"acc{ci}")
                for ci, (hb, hn) in enumerate(HCHUNKS)]

        bcs = [pool.tile([NDELTA, S, S], OFF_DT, name=f"bc{ds}_{ax}",
                         tag=f"bc{ax}", bufs=1) for ax in range(3)]
        hats = None
        first_mm = [True] * len(HCHUNKS)
        for gi, taps in enumerate(TAP_GROUPS):
            zetas = {}
            for k in taps:
                z = pool.tile([NDELTA, S, S], BF16, name=f"z{ds}_{k}",
                              tag=f"z{k % 14}")
                _build_zeta(nc, pool, ds, k, off_dram, dvecs, z, hats, bcs)
                zetas[k] = z
            last = (gi == len(TAP_GROUPS) - 1)
            for c in range(C):
                xr = pool.tile([NDELTA, 3, HP, WP], BF16,
                               name=f"xr{ds}_{gi}_{c}", tag="xr", bufs=2)
                xrf = xr.rearrange("p a h w -> p (a h w)")
                for a5 in range(5):
                    src = _win_ap(
                        xbf_d[c:c + 1, :],
                        c * NPAD + (dpad - 3 + a5) * HWP - 2 * WP - 2,
                        [(WP, 5), (1, 5), (1, 3 * HWP)])
                    nc.sync.dma_start(xrf[25 * a5:25 * a5 + 25, :], src)
                for k in taps:
                    kd, kh, kw = k // 9, (k // 3) % 3, k % 3
                    win = xr[:, kd, 3 + kh:3 + kh + S, 3 + kw:3 + kw + S]
                    p = pool.tile([NDELTA, S, S], BF16,
                                  name=f"p{ds}_{gi}_{c}_{k}", tag="ptile",
                                  bufs=2)
                    nc.vector.tensor_tensor(p[:], zetas[k][:], win, ALU.mult)
                    wsl = wdc_s[0:NDELTA, (k * C + c) * O:(k * C + c + 1) * O]
                    fin = last and (c == C - 1) and (k == taps[-1])
                    for ci, (hb, hn) in enumerate(HCHUNKS):
                        nc.tensor.matmul(accs[ci][:], wsl,
                                         p[:, hb:hb + hn, :],
                                         start=first_mm[ci], stop=fin)
                        first_mm[ci] = False

        # ---------------- evict ----------------
        outp = pool.tile([O, S, S], BF16, name=f"outp{ds}", tag="outp")
        for ci, (hb, hn) in enumerate(HCHUNKS):
            nc.vector.tensor_scalar(outp[:, hb:hb + hn, :], accs[ci][:],
                                    bdc_s[:, :], None, ALU.add)
        nc.sync.dma_start(out_d[:, ds * S * S:(ds + 1) * S * S],
                          outp.rearrange("p h w -> p (h w)"))


# ---------------------------------------------------------------- entry
def kernel(x, w_off, b_off, w_dc, b_dc):
    x = np.asarray(x, np.float32)
    w_off = np.asarray(w_off, np.float32)
    b_off = np.asarray(b_off, np.float32)
    w_dc = np.asarray(w_dc, np.float32)
    b_dc = np.asarray(b_dc, np.float32)

    in_maps = [_build_core_inputs(x, w_off, b_off, w_dc, b_dc,
                                  core // 4, core % 4) for core in range(8)]

    nc = bacc.Bacc("TRN2", target_bir_lowering=False, debug=False,
                   enable_asserts=False, num_devices=8)
    build_kernel(nc)
    if not nc.is_finalized():
        nc.finalize()

    global LAST_RESULTS
    LAST_RESULTS = run_bass_kernel_spmd(nc, in_maps, list(range(8)))
    res = LAST_RESULTS.results

    out = np.zeros((B, O, S, S, S), np.float32)
    for core in range(8):
        b, q = core // 4, core % 4
        out[b, :, DP * q:DP * q + NS_LOOP] = \
            res[core]["out"].reshape(O, NS_LOOP, S, S).astype(np.float32)
    return out



# revision 11
# speedup vs baseline: 1.0569x; 1.0569x over previous
"""Deformable 3D conv net on 8 Trainium2 NeuronCores (Bass/Tile).

Sharding: core (b, q) = batch b in {0,1} x D-quarter q in {0..3};
each core computes out[b, :, 12q:12q+12, :, :] from a padded x slab.

Per-core algorithm (exact trilinear, 5-wide window, exact for |off|<=2;
offsets clamped to [-2,2] on device; actual max |off| ~ 1.83):
  1. PE off-conv: off[81, 48,48] per d-slice, contraction K=96
     (3 w-shift replicas x 32 channels) accumulated over 9 (kd,kh) taps.
  2. Per tap k: zeta[(dd,dh,dw), h, w] = hat(od-dd)hat(oh-dh)hat(ow-dw)
     built with doubling copies + tensor_scalar chains (bf16, 125 rows).
  3. Per channel c: xr = 125 delta-shifted replicas of padded x channel c
     (one replicating DMA, double-buffered so c+1's load hides under c's
     compute); per tap: P = zeta * xr_window (DVE bf16); PE matmul K=125
     with stationary w_dc[o,c,k] broadcast over rows accumulates
     out[32, h, w] in PSUM across all (k, c).

Perf notes: offsets round-trip DRAM in bf16 (halves the 125-row zeta
broadcast-read traffic); hat tensors are computed in-place in the three
per-axis bc tiles (saves SBUF for xr double-buffering); output is
evicted bf16 and widened to f32 on host.
"""

import numpy as np
import ml_dtypes

import concourse.bass as bass
import concourse.bacc as bacc
import concourse.mybir as mybir
from concourse.tile import TileContext
from concourse.bass_utils import run_bass_kernel_spmd

B, C, O, S = 2, 32, 32, 48
KS, KV = 3, 27
PAD = 4
DP = 12                 # output D per core
DPP = DP + 2 * PAD      # 20
HP = WP = S + 2 * PAD   # 56
HWP = HP * WP           # 3136
NPAD = DPP * HWP        # 62720
NDELTA = 125

F32 = mybir.dt.float32
BF16 = mybir.dt.bfloat16
ALU = mybir.AluOpType
ACTF = mybir.ActivationFunctionType

TAP_GROUPS = [list(range(0, 14)), list(range(14, 27))]
HCHUNKS = [(0, 10), (10, 10), (20, 10), (30, 10), (40, 8)]  # h-row chunks
NS_LOOP = DP  # number of d-slices traced (reduce for simulation tests)
LAST_RESULTS = None


# ---------------------------------------------------------------- host prep
def _build_core_inputs(x, w_off, b_off, w_dc, b_dc, b, q):
    xp = np.zeros((C, DPP, HP, WP), np.float32)
    d0 = DP * q - PAD
    lo, hi = max(0, -d0), min(DPP, S - d0)
    xp[:, lo:hi, PAD:PAD + S, PAD:PAD + S] = x[b, :, d0 + lo:d0 + hi]

    # x3[32g+c, d, h, w] = xp[c, d, h, w + (g-1)]  (wrap lands in zero pad)
    x3 = np.zeros((96, DPP, HP, WP), np.float32)
    for g in range(3):
        x3[32 * g:32 * g + 32] = np.roll(xp, -(g - 1), axis=3)
    x3 = x3.reshape(96, NPAD).astype(ml_dtypes.bfloat16)

    x_bf = xp.reshape(C, NPAD).astype(ml_dtypes.bfloat16)

    # w_off9: [9*96, 81]: chunk (kd,kh), rows (kw, c), cols m = 3k + axis
    woff = w_off.reshape(KV, 3, C, KS, KS, KS)
    w_off9 = np.zeros((9, 96, 81), np.float32)
    for kd in range(3):
        for kh in range(3):
            ch = kd * 3 + kh
            for kw in range(3):
                blk = woff[:, :, :, kd, kh, kw]          # (k, ax, c)
                w_off9[ch, 32 * kw:32 * kw + 32, :] = \
                    blk.transpose(2, 0, 1).reshape(C, KV * 3)
    w_off9 = w_off9.astype(ml_dtypes.bfloat16)

    # wdc_rep: [128, KV*C*O]: rows = delta (125 used), free (k, c, o)
    wdcf = w_dc.reshape(O, C, KV)
    wdc = np.zeros((128, KV * C * O), np.float32)
    wdc[:NDELTA, :] = wdcf.transpose(2, 1, 0).reshape(KV * C * O)[None, :]
    wdc = wdc.astype(ml_dtypes.bfloat16)

    dd = np.repeat(np.arange(-2, 3), 25).astype(np.float32)[:, None]
    dh = np.tile(np.repeat(np.arange(-2, 3), 5), 5).astype(np.float32)[:, None]
    dw = np.tile(np.arange(-2, 3), 25).astype(np.float32)[:, None]

    return {
        "x3": np.ascontiguousarray(x3),
        "x_bf": np.ascontiguousarray(x_bf),
        "w_off9": np.ascontiguousarray(w_off9.transpose(1, 0, 2).reshape(96, 9 * 81)),
        "wdc_rep": np.ascontiguousarray(wdc),
        "b_off": np.ascontiguousarray(b_off.astype(np.float32).reshape(81, 1)),
        "b_dc": np.ascontiguousarray(b_dc.astype(np.float32).reshape(32, 1)),
        "dvec_d": dd, "dvec_h": dh, "dvec_w": dw,
    }


# ---------------------------------------------------------------- device IR
def _win_ap(dram_row_ap, offset, ap_dims):
    a = dram_row_ap.copy()
    a.ap = mybir.VecI64Pair(ap_dims)
    a.offset = offset
    return a


def _build_zeta(nc, pool, ds, k, off_dram, dvecs, zeta, hats, bcs):
    """hat(ax) = max(1 - |off_row - dvec|, 0) on 125 rows; zeta = prod."""
    for ax in range(3):
        bc = bcs[ax]
        src = _win_ap(off_dram[0:1, :],
                      (ds * 81 + 3 * k + ax) * S * S,
                      [(0, NDELTA), (1, S * S)])
        nc.sync.dma_start(bc.rearrange("p h w -> p (h w)"), src)
        # u = |dvec - bc| ; h = relu(1 - u)   (both on the scalar engine)
        nc.scalar.activation(bc[:], bc[:], ACTF.Abs,
                             bias=dvecs[ax][:, :], scale=-1.0)
        nc.scalar.activation(bc[:], bc[:], ACTF.Relu,
                             bias=1.0, scale=-1.0)
    nc.vector.tensor_tensor(zeta[:], bcs[0][:], bcs[1][:], ALU.mult)
    nc.vector.tensor_tensor(zeta[:], zeta[:], bcs[2][:], ALU.mult)


OFF_DT = BF16  # offsets round-trip DRAM in bf16: halves the zeta DMA bytes


def build_kernel(nc: bass.Bass):
    x3_d = nc.dram_tensor("x3", [96, NPAD], BF16, kind="ExternalInput")
    xbf_d = nc.dram_tensor("x_bf", [C, NPAD], BF16, kind="ExternalInput")
    woff_d = nc.dram_tensor("w_off9", [96, 9 * 81], BF16, kind="ExternalInput")
    wdc_d = nc.dram_tensor("wdc_rep", [128, KV * C * O], BF16,
                           kind="ExternalInput")
    boff_d = nc.dram_tensor("b_off", [81, 1], F32, kind="ExternalInput")
    bdc_d = nc.dram_tensor("b_dc", [32, 1], F32, kind="ExternalInput")
    dv_d = [nc.dram_tensor(n, [NDELTA, 1], F32, kind="ExternalInput")
            for n in ("dvec_d", "dvec_h", "dvec_w")]
    off_dram = nc.dram_tensor("off_scratch", [1, NS_LOOP * 81 * S * S], OFF_DT,
                              kind="Internal")
    out_d = nc.dram_tensor("out", [O, NS_LOOP * S * S], BF16,
                           kind="ExternalOutput")

    with TileContext(nc) as tc:
        with tc.tile_pool(name="fixed", bufs=1) as fixed:
            woff_s = fixed.tile([96, 9 * 81], BF16)
            nc.sync.dma_start(woff_s[:, :], woff_d[:, :])
            wdc_s = fixed.tile([128, KV * C * O], BF16)
            nc.sync.dma_start(wdc_s[:, :], wdc_d[:, :])
            boff_s = fixed.tile([81, 1], F32)
            nc.sync.dma_start(boff_s[:, :], boff_d[:, :])
            bdc_s = fixed.tile([32, 1], F32)
            nc.sync.dma_start(bdc_s[:, :], bdc_d[:, :])
            dvecs = []
            for i, t in enumerate(dv_d):
                dv = fixed.tile([NDELTA, 1], F32, name=f"dv{i}")
                nc.sync.dma_start(dv[:, :], t[:, :])
                dvecs.append(dv)

            # warm fixed tiles on DVE once so later DVE instructions don't
            # each carry a DMA-sem wait (HW wait-slot limit)
            warm = fixed.tile([1, 8], F32)
            for wsrc in [boff_s, bdc_s] + dvecs:
                nc.vector.tensor_copy(warm[0:1, 0:1], wsrc[0:1, 0:1])

            for ds in range(NS_LOOP):
                _do_offconv(nc, tc, ds, x3_d, off_dram, woff_s, boff_s)
            for ds in range(NS_LOOP):
                _do_slice(nc, tc, ds, x3_d, xbf_d, out_d, off_dram,
                          woff_s, wdc_s, boff_s, bdc_s, dvecs)
    return nc


def _do_offconv(nc, tc, ds, x3_d, off_dram, woff_s, boff_s):
    dpad = ds + PAD
    with tc.tile_pool(name=f"oc{ds}", bufs=1) as pool, \
         tc.tile_pool(name=f"ocps{ds}", bufs=1, space="PSUM") as psp:
        x3s = pool.tile([96, 3, HP, WP], BF16, name=f"x3s{ds}", tag="x3s",
                        bufs=2)
        nc.sync.dma_start(
            x3s.rearrange("p a h w -> p (a h w)"),
            x3_d[:, (dpad - 1) * HWP:(dpad + 2) * HWP])
        off = pool.tile([81, S, S], OFF_DT, name=f"off{ds}", tag="off",
                        bufs=2)
        for hc, (hb, hn) in enumerate(HCHUNKS):
            ps = psp.tile([81, hn, S], F32, name=f"offps{ds}_{hc}",
                          tag="offps", bufs=2)
            for i in range(9):
                kd, kh = i // 3, i % 3
                rhs = x3s[:, kd, 3 + kh + hb:3 + kh + hb + hn, 4:52]
                nc.tensor.matmul(ps[:], woff_s[:, i * 81:(i + 1) * 81],
                                 rhs, start=(i == 0), stop=(i == 8))
            # evict + bias + clamp to [-2, 2]
            nc.vector.tensor_scalar(off[:, hb:hb + hn, :], ps[:],
                                    boff_s[:, :], 2.0, ALU.add, ALU.min)
        nc.vector.tensor_scalar(off[:], off[:], -2.0, None, ALU.max)
        nc.sync.dma_start(
            _win_ap(off_dram[0:1, :], ds * 81 * S * S,
                    [(S * S, 81), (1, S * S)]),
            off.rearrange("p h w -> p (h w)"))


def _do_slice(nc, tc, ds, x3_d, xbf_d, out_d, off_dram,
              woff_s, wdc_s, boff_s, bdc_s, dvecs):
    dpad = ds + PAD
    with tc.tile_pool(name=f"sl{ds}", bufs=1) as pool, \
         tc.tile_pool(name=f"psum{ds}", bufs=1, space="PSUM") as psp:
        # ---------------- accumulators ----------------
        accs = [psp.tile([O, hn, S], F32, name=f"acc{ds}_{ci}", tag=f"acc{ci}")
                for ci, (hb, hn) in enumerate(HCHUNKS)]

        bcs = [pool.tile([NDELTA, S, S], OFF_DT, name=f"bc{ds}_{ax}",
                         tag=f"bc{ax}", bufs=1) for ax in range(3)]
        hats = None
        first_mm = [True] * len(HCHUNKS)
        for gi, taps in enumerate(TAP_GROUPS):
            zetas = {}
            for k in taps:
                z = pool.tile([NDELTA, S, S], BF16, name=f"z{ds}_{k}",
                              tag=f"z{k % 14}")
                _build_zeta(nc, pool, ds, k, off_dram, dvecs, z, hats, bcs)
                zetas[k] = z
            last = (gi == len(TAP_GROUPS) - 1)
            for c in range(C):
                xr = pool.tile([NDELTA, 3, HP, WP], BF16,
                               name=f"xr{ds}_{gi}_{c}", tag="xr", bufs=2)
                xrf = xr.rearrange("p a h w -> p (a h w)")
                xr_eng = [nc.sync, nc.gpsimd, nc.sync, nc.gpsimd,
                          nc.scalar]
                for a5 in range(5):
                    src = _win_ap(
                        xbf_d[c:c + 1, :],
                        c * NPAD + (dpad - 3 + a5) * HWP - 2 * WP - 2,
                        [(WP, 5), (1, 5), (1, 3 * HWP)])
                    xr_eng[a5].dma_start(xrf[25 * a5:25 * a5 + 25, :], src)
                for k in taps:
                    kd, kh, kw = k // 9, (k // 3) % 3, k % 3
                    win = xr[:, kd, 3 + kh:3 + kh + S, 3 + kw:3 + kw + S]
                    p = pool.tile([NDELTA, S, S], BF16,
                                  name=f"p{ds}_{gi}_{c}_{k}", tag="ptile",
                                  bufs=2)
                    nc.vector.tensor_tensor(p[:], zetas[k][:], win, ALU.mult)
                    wsl = wdc_s[0:NDELTA, (k * C + c) * O:(k * C + c + 1) * O]
                    fin = last and (c == C - 1) and (k == taps[-1])
                    for ci, (hb, hn) in enumerate(HCHUNKS):
                        nc.tensor.matmul(accs[ci][:], wsl,
                                         p[:, hb:hb + hn, :],
                                         start=first_mm[ci], stop=fin)
                        first_mm[ci] = False

        # ---------------- evict ----------------
        outp = pool.tile([O, S, S], BF16, name=f"outp{ds}", tag="outp")
        for ci, (hb, hn) in enumerate(HCHUNKS):
            nc.vector.tensor_scalar(outp[:, hb:hb + hn, :], accs[ci][:],
                                    bdc_s[:, :], None, ALU.add)
        nc.sync.dma_start(out_d[:, ds * S * S:(ds + 1) * S * S],
                          outp.rearrange("p h w -> p (h w)"))


# ---------------------------------------------------------------- entry
def kernel(x, w_off, b_off, w_dc, b_dc):
    x = np.asarray(x, np.float32)
    w_off = np.asarray(w_off, np.float32)
    b_off = np.asarray(b_off, np.float32)
    w_dc = np.asarray(w_dc, np.float32)
    b_dc = np.asarray(b_dc, np.float32)

    in_maps = [_build_core_inputs(x, w_off, b_off, w_dc, b_dc,
                                  core // 4, core % 4) for core in range(8)]

    nc = bacc.Bacc("TRN2", target_bir_lowering=False, debug=False,
                   enable_asserts=False, num_devices=8)
    build_kernel(nc)
    if not nc.is_finalized():
        nc.finalize()

    global LAST_RESULTS
    LAST_RESULTS = run_bass_kernel_spmd(nc, in_maps, list(range(8)))
    res = LAST_RESULTS.results

    out = np.zeros((B, O, S, S, S), np.float32)
    for core in range(8):
        b, q = core // 4, core % 4
        out[b, :, DP * q:DP * q + NS_LOOP] = \
            res[core]["out"].reshape(O, NS_LOOP, S, S).astype(np.float32)
    return out



# revision 12
# speedup vs baseline: 1.1223x; 1.0619x over previous
"""Deformable 3D conv net on 8 Trainium2 NeuronCores (Bass/Tile).

Sharding: core (b, q) = batch b in {0,1} x D-quarter q in {0..3};
each core computes out[b, :, 12q:12q+12, :, :] from a padded x slab.

Per-core algorithm (exact trilinear, 5-wide window, exact for |off|<=2;
offsets clamped to [-2,2] on device; actual max |off| ~ 1.83):
  1. PE off-conv: off[81, 48,48] per d-slice, contraction K=96
     (3 w-shift replicas x 32 channels) accumulated over 9 (kd,kh) taps.
  2. Per tap k: zeta[(dd,dh,dw), h, w] = hat(od-dd)hat(oh-dh)hat(ow-dw)
     built with doubling copies + tensor_scalar chains (bf16, 125 rows).
  3. Per channel c: xr = 125 delta-shifted replicas of padded x channel c
     (one replicating DMA, double-buffered so c+1's load hides under c's
     compute); per tap: P = zeta * xr_window (DVE bf16); PE matmul K=125
     with stationary w_dc[o,c,k] broadcast over rows accumulates
     out[32, h, w] in PSUM across all (k, c).

Perf notes: offsets round-trip DRAM in bf16 (halves the 125-row zeta
broadcast-read traffic); hat tensors are computed in-place in the three
per-axis bc tiles (saves SBUF for xr double-buffering); output is
evicted bf16 and widened to f32 on host.
"""

import numpy as np
import ml_dtypes

import concourse.bass as bass
import concourse.bacc as bacc
import concourse.mybir as mybir
from concourse.tile import TileContext
from concourse.bass_utils import run_bass_kernel_spmd

B, C, O, S = 2, 32, 32, 48
KS, KV = 3, 27
PAD = 4
DP = 12                 # output D per core
DPP = DP + 2 * PAD      # 20
HP = WP = S + 2 * PAD   # 56
HWP = HP * WP           # 3136
NPAD = DPP * HWP        # 62720
NDELTA = 125

F32 = mybir.dt.float32
BF16 = mybir.dt.bfloat16
ALU = mybir.AluOpType
ACTF = mybir.ActivationFunctionType

TAP_GROUPS = [list(range(0, 14)), list(range(14, 27))]
HCHUNKS = [(0, 10), (10, 10), (20, 10), (30, 10), (40, 8)]  # h-row chunks
NS_LOOP = DP  # number of d-slices traced (reduce for simulation tests)
LAST_RESULTS = None


# ---------------------------------------------------------------- host prep
def _build_core_inputs(x, w_off, b_off, w_dc, b_dc, b, q):
    xp = np.zeros((C, DPP, HP, WP), np.float32)
    d0 = DP * q - PAD
    lo, hi = max(0, -d0), min(DPP, S - d0)
    xp[:, lo:hi, PAD:PAD + S, PAD:PAD + S] = x[b, :, d0 + lo:d0 + hi]

    # x3[32g+c, d, h, w] = xp[c, d, h, w + (g-1)]  (wrap lands in zero pad)
    x3 = np.zeros((96, DPP, HP, WP), np.float32)
    for g in range(3):
        x3[32 * g:32 * g + 32] = np.roll(xp, -(g - 1), axis=3)
    x3 = x3.reshape(96, NPAD).astype(ml_dtypes.bfloat16)

    x_bf = xp.reshape(C, NPAD).astype(ml_dtypes.bfloat16)

    # w_off9: [9*96, 81]: chunk (kd,kh), rows (kw, c), cols m = 3k + axis
    woff = w_off.reshape(KV, 3, C, KS, KS, KS)
    w_off9 = np.zeros((9, 96, 81), np.float32)
    for kd in range(3):
        for kh in range(3):
            ch = kd * 3 + kh
            for kw in range(3):
                blk = woff[:, :, :, kd, kh, kw]          # (k, ax, c)
                w_off9[ch, 32 * kw:32 * kw + 32, :] = \
                    blk.transpose(2, 0, 1).reshape(C, KV * 3)
    w_off9 = w_off9.astype(ml_dtypes.bfloat16)

    # wdc_rep: [128, KV*C*O]: rows = delta (125 used), free (k, c, o)
    wdcf = w_dc.reshape(O, C, KV)
    wdc = np.zeros((128, KV * C * O), np.float32)
    wdc[:NDELTA, :] = wdcf.transpose(2, 1, 0).reshape(KV * C * O)[None, :]
    wdc = wdc.astype(ml_dtypes.bfloat16)

    dd = np.repeat(np.arange(-2, 3), 25).astype(np.float32)[:, None]
    dh = np.tile(np.repeat(np.arange(-2, 3), 5), 5).astype(np.float32)[:, None]
    dw = np.tile(np.arange(-2, 3), 25).astype(np.float32)[:, None]

    return {
        "x3": np.ascontiguousarray(x3),
        "x_bf": np.ascontiguousarray(x_bf),
        "w_off9": np.ascontiguousarray(w_off9.transpose(1, 0, 2).reshape(96, 9 * 81)),
        "wdc_rep": np.ascontiguousarray(wdc),
        "b_off": np.ascontiguousarray(b_off.astype(np.float32).reshape(81, 1)),
        "b_dc": np.ascontiguousarray(b_dc.astype(np.float32).reshape(32, 1)),
        "dvec_d": dd, "dvec_h": dh, "dvec_w": dw,
    }


# ---------------------------------------------------------------- device IR
def _win_ap(dram_row_ap, offset, ap_dims):
    a = dram_row_ap.copy()
    a.ap = mybir.VecI64Pair(ap_dims)
    a.offset = offset
    return a


def _build_zeta(nc, pool, ds, k, off_dram, dvecs, zeta, hats, bcs):
    """hat(ax) = max(1 - |off_row - dvec|, 0) on 125 rows; zeta = prod."""
    for ax in range(3):
        bc = bcs[ax]
        src = _win_ap(off_dram[0:1, :],
                      (ds * 81 + 3 * k + ax) * S * S,
                      [(0, NDELTA), (1, S * S)])
        nc.sync.dma_start(bc.rearrange("p h w -> p (h w)"), src)
        # u = |dvec - bc| ; h = relu(1 - u)   (both on the scalar engine)
        nc.scalar.activation(bc[:], bc[:], ACTF.Abs,
                             bias=dvecs[ax][:, :], scale=-1.0)
        nc.scalar.activation(bc[:], bc[:], ACTF.Relu,
                             bias=1.0, scale=-1.0)
    nc.vector.tensor_tensor(zeta[:], bcs[0][:], bcs[1][:], ALU.mult)
    nc.vector.tensor_tensor(zeta[:], zeta[:], bcs[2][:], ALU.mult)


OFF_DT = BF16  # offsets round-trip DRAM in bf16: halves the zeta DMA bytes


def build_kernel(nc: bass.Bass):
    x3_d = nc.dram_tensor("x3", [96, NPAD], BF16, kind="ExternalInput")
    xbf_d = nc.dram_tensor("x_bf", [C, NPAD], BF16, kind="ExternalInput")
    woff_d = nc.dram_tensor("w_off9", [96, 9 * 81], BF16, kind="ExternalInput")
    wdc_d = nc.dram_tensor("wdc_rep", [128, KV * C * O], BF16,
                           kind="ExternalInput")
    boff_d = nc.dram_tensor("b_off", [81, 1], F32, kind="ExternalInput")
    bdc_d = nc.dram_tensor("b_dc", [32, 1], F32, kind="ExternalInput")
    dv_d = [nc.dram_tensor(n, [NDELTA, 1], F32, kind="ExternalInput")
            for n in ("dvec_d", "dvec_h", "dvec_w")]
    off_dram = nc.dram_tensor("off_scratch", [1, NS_LOOP * 81 * S * S], OFF_DT,
                              kind="Internal")
    out_d = nc.dram_tensor("out", [O, NS_LOOP * S * S], BF16,
                           kind="ExternalOutput")

    with TileContext(nc) as tc:
        with tc.tile_pool(name="fixed", bufs=1) as fixed:
            woff_s = fixed.tile([96, 9 * 81], BF16)
            nc.sync.dma_start(woff_s[:, :], woff_d[:, :])
            wdc_s = fixed.tile([128, KV * C * O], BF16)
            nc.sync.dma_start(wdc_s[:, :], wdc_d[:, :])
            boff_s = fixed.tile([81, 1], F32)
            nc.sync.dma_start(boff_s[:, :], boff_d[:, :])
            bdc_s = fixed.tile([32, 1], F32)
            nc.sync.dma_start(bdc_s[:, :], bdc_d[:, :])
            dvecs = []
            for i, t in enumerate(dv_d):
                dv = fixed.tile([NDELTA, 1], F32, name=f"dv{i}")
                nc.sync.dma_start(dv[:, :], t[:, :])
                dvecs.append(dv)

            # warm fixed tiles on DVE once so later DVE instructions don't
            # each carry a DMA-sem wait (HW wait-slot limit)
            warm = fixed.tile([1, 8], F32)
            for wsrc in [boff_s, bdc_s] + dvecs:
                nc.vector.tensor_copy(warm[0:1, 0:1], wsrc[0:1, 0:1])

            for ds in range(NS_LOOP):
                _do_offconv(nc, tc, ds, x3_d, off_dram, woff_s, boff_s)
            for ds in range(NS_LOOP):
                _do_slice(nc, tc, ds, x3_d, xbf_d, out_d, off_dram,
                          woff_s, wdc_s, boff_s, bdc_s, dvecs)
    return nc


def _do_offconv(nc, tc, ds, x3_d, off_dram, woff_s, boff_s):
    dpad = ds + PAD
    with tc.tile_pool(name=f"oc{ds}", bufs=1) as pool, \
         tc.tile_pool(name=f"ocps{ds}", bufs=1, space="PSUM") as psp:
        x3s = pool.tile([96, 3, HP, WP], BF16, name=f"x3s{ds}", tag="x3s",
                        bufs=2)
        nc.sync.dma_start(
            x3s.rearrange("p a h w -> p (a h w)"),
            x3_d[:, (dpad - 1) * HWP:(dpad + 2) * HWP])
        off = pool.tile([81, S, S], OFF_DT, name=f"off{ds}", tag="off",
                        bufs=2)
        for hc, (hb, hn) in enumerate(HCHUNKS):
            ps = psp.tile([81, hn, S], F32, name=f"offps{ds}_{hc}",
                          tag="offps", bufs=2)
            for i in range(9):
                kd, kh = i // 3, i % 3
                rhs = x3s[:, kd, 3 + kh + hb:3 + kh + hb + hn, 4:52]
                nc.tensor.matmul(ps[:], woff_s[:, i * 81:(i + 1) * 81],
                                 rhs, start=(i == 0), stop=(i == 8))
            # evict + bias + clamp to [-2, 2]
            nc.vector.tensor_scalar(off[:, hb:hb + hn, :], ps[:],
                                    boff_s[:, :], 2.0, ALU.add, ALU.min)
        nc.vector.tensor_scalar(off[:], off[:], -2.0, None, ALU.max)
        nc.sync.dma_start(
            _win_ap(off_dram[0:1, :], ds * 81 * S * S,
                    [(S * S, 81), (1, S * S)]),
            off.rearrange("p h w -> p (h w)"))


def _do_slice(nc, tc, ds, x3_d, xbf_d, out_d, off_dram,
              woff_s, wdc_s, boff_s, bdc_s, dvecs):
    dpad = ds + PAD
    with tc.tile_pool(name=f"sl{ds}", bufs=1) as pool, \
         tc.tile_pool(name=f"psum{ds}", bufs=1, space="PSUM") as psp:
        # ---------------- accumulators ----------------
        accs = [psp.tile([O, hn, S], F32, name=f"acc{ds}_{ci}", tag=f"acc{ci}")
                for ci, (hb, hn) in enumerate(HCHUNKS)]

        bcs = [pool.tile([NDELTA, S, S], OFF_DT, name=f"bc{ds}_{ax}",
                         tag=f"bc{ax}", bufs=1) for ax in range(3)]
        hats = None
        first_mm = [True] * len(HCHUNKS)
        for gi, taps in enumerate(TAP_GROUPS):
            zetas = {}
            for k in taps:
                z = pool.tile([NDELTA, S, S], BF16, name=f"z{ds}_{k}",
                              tag=f"z{k % 14}")
                _build_zeta(nc, pool, ds, k, off_dram, dvecs, z, hats, bcs)
                zetas[k] = z
            last = (gi == len(TAP_GROUPS) - 1)
            for c in range(C):
                xr = pool.tile([NDELTA, 3, HP, WP], BF16,
                               name=f"xr{ds}_{gi}_{c}", tag="xr", bufs=3)
                xrf = xr.rearrange("p a h w -> p (a h w)")
                xr_eng = [nc.sync, nc.gpsimd, nc.sync, nc.gpsimd,
                          nc.scalar]
                for a5 in range(5):
                    src = _win_ap(
                        xbf_d[c:c + 1, :],
                        c * NPAD + (dpad - 3 + a5) * HWP - 2 * WP - 2,
                        [(WP, 5), (1, 5), (1, 3 * HWP)])
                    xr_eng[a5].dma_start(xrf[25 * a5:25 * a5 + 25, :], src)
                for k in taps:
                    kd, kh, kw = k // 9, (k // 3) % 3, k % 3
                    win = xr[:, kd, 3 + kh:3 + kh + S, 3 + kw:3 + kw + S]
                    p = pool.tile([NDELTA, S, S], BF16,
                                  name=f"p{ds}_{gi}_{c}_{k}", tag="ptile",
                                  bufs=3)
                    nc.vector.tensor_tensor(p[:], zetas[k][:], win, ALU.mult)
                    wsl = wdc_s[0:NDELTA, (k * C + c) * O:(k * C + c + 1) * O]
                    fin = last and (c == C - 1) and (k == taps[-1])
                    for ci, (hb, hn) in enumerate(HCHUNKS):
                        nc.tensor.matmul(accs[ci][:], wsl,
                                         p[:, hb:hb + hn, :],
                                         start=first_mm[ci], stop=fin)
                        first_mm[ci] = False

        # ---------------- evict ----------------
        outp = pool.tile([O, S, S], BF16, name=f"outp{ds}", tag="outp")
        for ci, (hb, hn) in enumerate(HCHUNKS):
            nc.vector.tensor_scalar(outp[:, hb:hb + hn, :], accs[ci][:],
                                    bdc_s[:, :], None, ALU.add)
        nc.sync.dma_start(out_d[:, ds * S * S:(ds + 1) * S * S],
                          outp.rearrange("p h w -> p (h w)"))


# ---------------------------------------------------------------- entry
def kernel(x, w_off, b_off, w_dc, b_dc):
    x = np.asarray(x, np.float32)
    w_off = np.asarray(w_off, np.float32)
    b_off = np.asarray(b_off, np.float32)
    w_dc = np.asarray(w_dc, np.float32)
    b_dc = np.asarray(b_dc, np.float32)

    in_maps = [_build_core_inputs(x, w_off, b_off, w_dc, b_dc,
                                  core // 4, core % 4) for core in range(8)]

    nc = bacc.Bacc("TRN2", target_bir_lowering=False, debug=False,
                   enable_asserts=False, num_devices=8)
    build_kernel(nc)
    if not nc.is_finalized():
        nc.finalize()

    global LAST_RESULTS
    LAST_RESULTS = run_bass_kernel_spmd(nc, in_maps, list(range(8)))
    res = LAST_RESULTS.results

    out = np.zeros((B, O, S, S, S), np.float32)
    for core in range(8):
        b, q = core // 4, core % 4
        out[b, :, DP * q:DP * q + NS_LOOP] = \
            res[core]["out"].reshape(O, NS_LOOP, S, S).astype(np.float32)
    return out

